# revision 14
# baseline (speedup 1.0000x reference)
"""Devign GGNN model on 8 Trainium2 NeuronCores (~321 us HW exec).

Strategy (data-parallel over graphs, 4 graphs/core):
- Edge gather + scatter-add replaced by dense per-(graph, edge-type)
  adjacency matmuls: a = sum_t A_t @ (h @ W_t.T). A_t is built host-side
  from the integer edge lists (small exact counts, fp8-e4m3).
- Nodes packed (4x513 = 2052 rows, padded to 17x128 = 2176); each graph's
  adjacency strip touches exactly 5 source chunks (513*g starts at chunk 4g),
  so block-diagonality costs no extra matmuls and no per-graph padding.
- Adjacency matmuls run in fp8 DoubleRow (2 contraction rows/cycle). A is
  exact in fp8; messages (hw) pay fp8 quantization. At step 0 — the dominant
  error source — the fp8 quantization residual of hw is kept and
  A@hw + A@hw_r is accumulated, recovering ~bf16 end-to-end accuracy.
- All state is SBUF-resident in "transposed" layouts (feature dim on
  partitions): zero on-device transposes. GRU gate matmuls accumulate
  W_ih@a and W_hh@h in one PSUM group (ir+hr fused); conv head consumes
  h_T/feat_T partition-chunks directly as the channel dimension.
- Everything else bf16 operands with fp32 PSUM accumulation.
"""

import os
import sys

for _p in ("/opt/trn_rl_repo",):
    if os.path.isdir(_p) and _p not in sys.path:
        sys.path.append(_p)

import numpy as np
import ml_dtypes

BF16 = ml_dtypes.bfloat16

B, NN, IN, OUT, T, STEPS = 32, 513, 128, 256, 4, 4
CAT = OUT + IN
NCORES = 8
GPC = B // NCORES          # graphs per core = 4
NV = GPC * NN              # valid packed rows per core = 2052
KCH = 17                   # packed row chunks (2176 = 17 x 128)
NP = KCH * 128             # padded packed rows = 2176
SKC = 5                    # src chunks per graph strip (graph g: chunks 4g..4g+4)
SC = SKC * T               # strip chunk count incl types = 20
SL = [(0, 512), (512, 1024), (1024, 1536), (1536, 2048), (2048, NP)]
ASL = [(0, 320), (320, NN)]  # adjacency dst sub-slabs per graph (513 cols)
L1, P1 = NN - 2, 255       # conv1 out len, pool1 out len
L2Y, P2 = P1, 127          # conv2(k=1) len, final pooled len
L2Z = P1 - 1               # convc2(k=2) out len = 254

USE_FP8_A = True  # fp8-e4m3 DoubleRow adjacency matmul (2x PE rate, more err)
_prog_cache = {}


def _build_program(flags):
    import concourse.bacc as bacc
    import concourse.mybir as mybir
    import concourse.tile as tile

    has_bmsg, has_gru_b, has_conv_b, has_mlp_b, use_fp8 = flags
    f32 = mybir.dt.float32
    bf16 = mybir.dt.bfloat16
    AF = mybir.ActivationFunctionType
    OP = mybir.AluOpType

    nc = bacc.Bacc("TRN2", target_bir_lowering=False, debug=False,
                   enable_asserts=False, num_devices=NCORES)
    hw_dt = mybir.dt.float8e4 if use_fp8 else bf16

    # ---- DRAM I/O (all pre-laid-out host side, partition dim first) ----
    d_feat = nc.dram_tensor("feat", [128, NP], bf16, kind="ExternalInput").ap()
    a_dt = mybir.dt.float8e4 if use_fp8 else bf16
    d_A = nc.dram_tensor("adj", [128, GPC, SC, NN], a_dt, kind="ExternalInput").ap()
    d_wmsg = nc.dram_tensor("wmsg", [128, 2, T, OUT], bf16, kind="ExternalInput").ap()
    d_wih = nc.dram_tensor("wih", [128, 2, 3 * OUT], bf16, kind="ExternalInput").ap()
    d_whh = nc.dram_tensor("whh", [128, 2, 3 * OUT], bf16, kind="ExternalInput").ap()
    d_c1w = nc.dram_tensor("c1w", [128, 3, 2, 2, 128], bf16, kind="ExternalInput").ap()
    d_c2w = nc.dram_tensor("c2w", [128, 1, 2, 2, 128], bf16, kind="ExternalInput").ap()
    d_cc1w = nc.dram_tensor("cc1w", [128, 3, 3, 3, 128], bf16, kind="ExternalInput").ap()
    d_cc2w = nc.dram_tensor("cc2w", [128, 2, 3, 3, 128], bf16, kind="ExternalInput").ap()
    d_mlpy = nc.dram_tensor("mlpy", [128, 2], bf16, kind="ExternalInput").ap()
    d_mlpz = nc.dram_tensor("mlpz", [128, 3], bf16, kind="ExternalInput").ap()
    if has_bmsg:
        d_bmsg = nc.dram_tensor("bmsg", [T, OUT], f32, kind="ExternalInput").ap()
        d_indeg = nc.dram_tensor("indeg", [T, NP], f32, kind="ExternalInput").ap()
    if has_gru_b:
        d_gbias = nc.dram_tensor("gbias", [128, 12], f32, kind="ExternalInput").ap()
    if has_conv_b:
        d_cbias = nc.dram_tensor("cbias", [128, 10], f32, kind="ExternalInput").ap()
    if has_mlp_b:
        d_mbias = nc.dram_tensor("mbias", [1, 2], f32, kind="ExternalInput").ap()
    d_out = nc.dram_tensor("out", [GPC], f32, kind="ExternalOutput").ap()

    def mm_acc(nct, ps, pairs):
        n = len(pairs)
        for i, (l, r) in enumerate(pairs):
            nct.tensor.matmul(ps, l, r, start=(i == 0), stop=(i == n - 1))

    with tile.TileContext(nc) as tc:
        from contextlib import ExitStack
        with ExitStack() as ctx:
            cpool = ctx.enter_context(tc.tile_pool(name="const", bufs=1))
            hpool = ctx.enter_context(tc.tile_pool(name="hstate", bufs=1))

            # ---- persistent tiles ----
            feat = cpool.tile([128, NP], bf16, tag="feat")
            wmsg = cpool.tile([128, 2, T, OUT], bf16, tag="wmsg")
            wih = cpool.tile([128, 2, 3 * OUT], bf16, tag="wih")
            whh = cpool.tile([128, 2, 3 * OUT], bf16, tag="whh")
            c1w = cpool.tile([128, 3, 2, 2, 128], bf16, tag="c1w")
            c2w = cpool.tile([128, 1, 2, 2, 128], bf16, tag="c2w")
            cc1w = cpool.tile([128, 3, 3, 3, 128], bf16, tag="cc1w")
            cc2w = cpool.tile([128, 2, 3, 3, 128], bf16, tag="cc2w")
            mlpy = cpool.tile([128, 2], bf16, tag="mlpy")
            mlpz = cpool.tile([128, 3], bf16, tag="mlpz")
            hT = [hpool.tile([128, NP], bf16, tag=f"hT{m}", name=f"hT{m}")
                  for m in range(2)]
            out_sb = cpool.tile([1, GPC], f32, tag="outsb")

            nc.sync.dma_start(out=feat[:], in_=d_feat[:])
            nc.sync.dma_start(out=wmsg[:], in_=d_wmsg[:])

            if has_conv_b:
                cbias = cpool.tile([128, 10], f32, tag="cbias")
                nc.sync.dma_start(out=cbias[:], in_=d_cbias[:])
            if has_mlp_b:
                mbias = cpool.tile([1, 2], f32, tag="mbias")
                nc.sync.dma_start(out=mbias[:], in_=d_mbias[:])

            # h0 = [feature | 0] is consumed in-place at step 0 (no copy);
            # hT tiles are first written by the step-0 GRU update.

            # ================= GGNN =================
            with ExitStack() as gctx:
                apool = gctx.enter_context(tc.tile_pool(name="adj", bufs=1))
                hwpool = gctx.enter_context(tc.tile_pool(name="hw", bufs=2))
                atpool = gctx.enter_context(tc.tile_pool(name="aT", bufs=1))
                grupool = gctx.enter_context(tc.tile_pool(name="gru", bufs=2))
                ps_hw = gctx.enter_context(
                    tc.tile_pool(name="pshw", bufs=2, space="PSUM"))
                ps_g = gctx.enter_context(
                    tc.tile_pool(name="psg", bufs=6, space="PSUM"))

                A_sb = apool.tile([128, GPC, SC, NN], a_dt, tag="A")
                for g in range(GPC):
                    nc.sync.dma_start(out=A_sb[:, g], in_=d_A[:, g])
                nc.sync.dma_start(out=wih[:], in_=d_wih[:])
                nc.sync.dma_start(out=whh[:], in_=d_whh[:])
                nc.sync.dma_start(out=c1w[:], in_=d_c1w[:])
                nc.sync.dma_start(out=c2w[:], in_=d_c2w[:])
                nc.sync.dma_start(out=cc1w[:], in_=d_cc1w[:])
                nc.sync.dma_start(out=cc2w[:], in_=d_cc2w[:])
                nc.sync.dma_start(out=mlpy[:], in_=d_mlpy[:])
                nc.sync.dma_start(out=mlpz[:], in_=d_mlpz[:])
                aT = atpool.tile([128, 2, NP], bf16, tag="aT")
                # adjacency writes only valid dst cols; zero the pad once
                nc.vector.memset(aT[:, :, NV:], 0.0)

                if has_bmsg:
                    bmsg = cpool.tile([T, OUT], f32, tag="bmsg")
                    indeg = cpool.tile([T, NP], f32, tag="indeg")
                    nc.sync.dma_start(out=bmsg[:], in_=d_bmsg[:])
                    nc.sync.dma_start(out=indeg[:], in_=d_indeg[:])
                    bias_a = [cpool.tile([128, NP], bf16, tag=f"biasa{m}",
                                         name=f"biasa{m}") for m in range(2)]
                    for m in range(2):
                        for (s0, s1) in SL:
                            ps = ps_g.tile([128, s1 - s0], f32, tag="psg",
                                           name="psb")
                            nc.tensor.matmul(
                                ps[:], bmsg[:, m * 128:(m + 1) * 128],
                                indeg[:, s0:s1], start=True, stop=True)
                            nc.vector.tensor_copy(
                                out=bias_a[m][:, s0:s1], in_=ps[:])
                if has_gru_b:
                    gbias = cpool.tile([128, 12], f32, tag="gbias")
                    nc.sync.dma_start(out=gbias[:], in_=d_gbias[:])
                    bias_rz = cpool.tile([128, 4], f32, tag="biasrz")
                    nc.vector.tensor_add(
                        out=bias_rz[:], in0=gbias[:, 0:4], in1=gbias[:, 6:10])

                for s in range(STEPS):
                    kr = 1 if s == 0 else 2
                    hsrc = [feat, None] if s == 0 else hT
                    # --- messages: fp8 path computes all 17 packed chunks
                    # once; bf16 path computes 5-chunk strips per graph (the
                    # 3 boundary chunks twice) to fit SBUF.
                    if use_fp8:
                        hw = hwpool.tile([128, KCH * T, 256], hw_dt, tag="hw")
                        # step 0: also keep the fp8 quantization residual and
                        # accumulate A@hw + A@hw_r (first-step messages are the
                        # dominant fp8 error source; this makes them ~bf16-exact)
                        hw_r = (hwpool.tile([128, KCH * T, 256], hw_dt, tag="hwr",
                                            name="hw_r") if s == 0 else None)
                        for rc in range(KCH):
                            for tp in range(2):
                                ps = ps_hw.tile([128, 512], f32, tag="pshw")
                                mm_acc(nc, ps[:], [
                                    (hsrc[k][:, rc * 128:(rc + 1) * 128],
                                     wmsg[:, k, 2 * tp: 2 * tp + 2, :])
                                    for k in range(kr)])
                                hsl = slice(rc * T + 2 * tp, rc * T + 2 * tp + 2)
                                nc.scalar.copy(out=hw[:, hsl, :], in_=ps[:])
                                if hw_r is not None:
                                    nc.vector.tensor_sub(
                                        out=hw_r[:, hsl, :], in0=ps[:],
                                        in1=hw[:, hsl, :])
                    # --- adjacency matmul per graph strip ---
                    for g in range(GPC):
                        base = g * NN
                        if not use_fp8:
                            hw_r = None
                            hw = hwpool.tile([128, SC, 256], hw_dt, tag="hw")
                            for kl in range(SKC):
                                rc = 4 * g + kl
                                for tp in range(2):
                                    ps = ps_hw.tile([128, 512], f32, tag="pshw")
                                    mm_acc(nc, ps[:], [
                                        (hsrc[k][:, rc * 128:(rc + 1) * 128],
                                         wmsg[:, k, 2 * tp: 2 * tp + 2, :])
                                        for k in range(kr)])
                                    nc.scalar.copy(
                                        out=hw[:, kl * T + 2 * tp:
                                               kl * T + 2 * tp + 2, :],
                                        in_=ps[:])
                        for m in range(2):
                            pa = [ps_g.tile([128, n1 - n0], f32, tag="psg",
                                            name=f"pa{n0}")
                                  for (n0, n1) in ASL]
                            for ps, (n0, n1) in zip(pa, ASL):
                                if use_fp8:
                                    hws = [hw] if hw_r is None else [hw, hw_r]
                                    nmm = (SC // 2) * len(hws)
                                    i = 0
                                    for hwt in hws:
                                        for i2 in range(SC // 2):
                                            nc.tensor.matmul(
                                                ps[:],
                                                hwt[:, 16 * g + 2 * i2:
                                                    16 * g + 2 * i2 + 2,
                                                    m * 128:(m + 1) * 128],
                                                A_sb[:, g, 2 * i2: 2 * i2 + 2, n0:n1],
                                                start=(i == 0), stop=(i == nmm - 1),
                                                perf_mode=mybir.MatmulPerfMode.DoubleRow)
                                            i += 1
                                else:
                                    mm_acc(nc, ps[:], [
                                        (hw[:, c, m * 128:(m + 1) * 128],
                                         A_sb[:, g, c, n0:n1])
                                        for c in range(SC)])
                            for ps, (n0, n1) in zip(pa, ASL):
                                if has_bmsg:
                                    nc.vector.tensor_add(
                                        out=aT[:, m, base + n0:base + n1], in0=ps[:],
                                        in1=bias_a[m][:, base + n0:base + n1])
                                else:
                                    nc.vector.tensor_copy(
                                        out=aT[:, m, base + n0:base + n1], in_=ps[:])
                    # --- GRU, per 512-row slab ---
                    for (s0, s1) in SL:
                        w = s1 - s0
                        cs = slice(s0, s1)
                        rz = grupool.tile([128, 4, 512], bf16, tag="rz", name="rz")[:, :, :w]
                        nt = grupool.tile([128, 2, 512], bf16, tag="nt", name="nt")[:, :, :w]
                        for gc in range(4):
                            # one PSUM group accumulates ir+hr (iz+hz)
                            ps = ps_g.tile([128, 512], f32, tag="psg", name="psgr")[:, :w]
                            mm_acc(nc, ps[:], [
                                (wih[:, k, gc * 128:(gc + 1) * 128], aT[:, k, cs])
                                for k in range(2)] + [
                                (whh[:, k, gc * 128:(gc + 1) * 128], hsrc[k][:, cs])
                                for k in range(kr)])
                            nc.scalar.activation(
                                rz[:, gc, :], ps[:], AF.Sigmoid,
                                bias=bias_rz[:, gc:gc + 1] if has_gru_b else 0.0)
                        for j in range(2):
                            gc = 4 + j
                            pi = ps_g.tile([128, 512], f32, tag="psg", name="pgi")[:, :w]
                            mm_acc(nc, pi[:], [
                                (wih[:, k, gc * 128:(gc + 1) * 128], aT[:, k, cs])
                                for k in range(2)])
                            ph = ps_g.tile([128, 512], f32, tag="psg", name="pgh")[:, :w]
                            mm_acc(nc, ph[:], [
                                (whh[:, k, gc * 128:(gc + 1) * 128], hsrc[k][:, cs])
                                for k in range(kr)])
                            if has_gru_b:
                                nc.vector.tensor_scalar_add(
                                    out=pi[:], in0=pi[:], scalar1=gbias[:, gc:gc + 1])
                                nc.vector.tensor_scalar_add(
                                    out=ph[:], in0=ph[:], scalar1=gbias[:, 6 + gc:7 + gc])
                            rhn = grupool.tile([128, 512], f32, tag="rhn", name="rhn")[:, :w]
                            nc.vector.tensor_tensor(
                                out=rhn[:], in0=rz[:, j, :], in1=ph[:], op=OP.mult)
                            nc.vector.tensor_add(out=pi[:], in0=pi[:], in1=rhn[:])
                            nc.scalar.activation(nt[:, j, :], pi[:], AF.Tanh)
                        for m in range(2):
                            d = grupool.tile([128, 512], f32, tag="d", name="d")[:, :w]
                            if s == 0 and m == 1:
                                # h=0: h' = n - z*n
                                nc.vector.tensor_tensor(
                                    out=d[:], in0=rz[:, 3, :], in1=nt[:, 1, :],
                                    op=OP.mult)
                                nc.vector.tensor_sub(
                                    out=hT[1][:, cs], in0=nt[:, 1, :], in1=d[:])
                                continue
                            nc.vector.tensor_sub(
                                out=d[:], in0=hsrc[m][:, cs], in1=nt[:, m, :])
                            nc.vector.tensor_tensor(
                                out=d[:], in0=rz[:, 2 + m, :], in1=d[:], op=OP.mult)
                            nc.vector.tensor_add(
                                out=hT[m][:, cs], in0=nt[:, m, :], in1=d[:])

            # ================= conv heads =================
            with ExitStack() as cctx:
                ypool = cctx.enter_context(tc.tile_pool(name="yact", bufs=2))
                zpool = cctx.enter_context(tc.tile_pool(name="zact", bufs=2))
                ps_c = cctx.enter_context(
                    tc.tile_pool(name="psc", bufs=6, space="PSUM"))

                xs = [hT[0], hT[1], feat]
                for g in range(GPC):
                    base = g * NN
                    # stage 1: all five conv1 output chunks (Y then Z) so PE
                    # has a long uninterrupted run while pools/relu trail
                    y1 = ypool.tile([128, 2, L1], bf16, tag="y1")
                    z1 = zpool.tile([128, 3, L1], bf16, tag="z1")
                    for co in range(2):
                        ps = ps_c.tile([128, L1], f32, tag="psc")
                        mm_acc(nc, ps[:], [
                            (c1w[:, k, ci, co, :], xs[ci][:, base + k: base + k + L1])
                            for k in range(3) for ci in range(2)])
                        nc.scalar.activation(
                            y1[:, co, :], ps[:], AF.Relu,
                            bias=cbias[:, co:co + 1] if has_conv_b else 0.0)
                    for co in range(3):
                        ps = ps_c.tile([128, L1], f32, tag="psc")
                        mm_acc(nc, ps[:], [
                            (cc1w[:, k, ci, co, :], xs[ci][:, base + k: base + k + L1])
                            for k in range(3) for ci in range(3)])
                        nc.scalar.activation(
                            z1[:, co, :], ps[:], AF.Relu,
                            bias=cbias[:, 4 + co:5 + co] if has_conv_b else 0.0)
                    # stage 2: pools
                    y1p = ypool.tile([128, 2, P1], bf16, tag="y1p")
                    z1p = zpool.tile([128, 3, P1], bf16, tag="z1p")
                    for co in range(2):
                        nc.vector.tensor_tensor(
                            out=y1p[:, co, :], in0=y1[:, co, 0:510:2],
                            in1=y1[:, co, 1:510:2], op=OP.max)
                        nc.vector.tensor_tensor(
                            out=y1p[:, co, :], in0=y1p[:, co, :],
                            in1=y1[:, co, 2:511:2], op=OP.max)
                    for co in range(3):
                        nc.vector.tensor_tensor(
                            out=z1p[:, co, :], in0=z1[:, co, 0:510:2],
                            in1=z1[:, co, 1:510:2], op=OP.max)
                        nc.vector.tensor_tensor(
                            out=z1p[:, co, :], in0=z1p[:, co, :],
                            in1=z1[:, co, 2:511:2], op=OP.max)
                    # stage 3: second convs
                    y2 = ypool.tile([128, 2, L2Y], bf16, tag="y2")
                    z2 = zpool.tile([128, 3, L2Z], bf16, tag="z2")
                    for co in range(2):
                        ps = ps_c.tile([128, L2Y], f32, tag="psc")
                        mm_acc(nc, ps[:], [
                            (c2w[:, 0, ci, co, :], y1p[:, ci, :]) for ci in range(2)])
                        nc.scalar.activation(
                            y2[:, co, :], ps[:], AF.Relu,
                            bias=cbias[:, 2 + co:3 + co] if has_conv_b else 0.0)
                    for co in range(3):
                        ps = ps_c.tile([128, L2Z], f32, tag="psc")
                        mm_acc(nc, ps[:], [
                            (cc2w[:, k, ci, co, :], z1p[:, ci, k:k + L2Z])
                            for k in range(2) for ci in range(3)])
                        nc.scalar.activation(
                            z2[:, co, :], ps[:], AF.Relu,
                            bias=cbias[:, 7 + co:8 + co] if has_conv_b else 0.0)
                    # stage 4: pools + heads
                    y2p = ypool.tile([128, 2, P2], bf16, tag="y2p")
                    z2p = zpool.tile([128, 3, P2], bf16, tag="z2p")
                    for co in range(2):
                        nc.vector.tensor_tensor(
                            out=y2p[:, co, :], in0=y2[:, co, 0:254:2],
                            in1=y2[:, co, 1:254:2], op=OP.max)
                    for co in range(3):
                        nc.vector.tensor_tensor(
                            out=z2p[:, co, :], in0=z2[:, co, 0:254:2],
                            in1=z2[:, co, 1:254:2], op=OP.max)
                    psy = ps_c.tile([1, P2], f32, tag="psc")
                    mm_acc(nc, psy[:], [
                        (mlpy[:, co:co + 1], y2p[:, co, :]) for co in range(2)])
                    ys = ypool.tile([1, P2], f32, tag="ys")
                    if has_mlp_b:
                        nc.vector.tensor_scalar_add(
                            out=ys[:], in0=psy[:], scalar1=mbias[:, 0:1])
                    else:
                        nc.vector.tensor_copy(out=ys[:], in_=psy[:])
                    psz = ps_c.tile([1, P2], f32, tag="psc")
                    mm_acc(nc, psz[:], [
                        (mlpz[:, co:co + 1], z2p[:, co, :]) for co in range(3)])
                    zs = zpool.tile([1, P2], f32, tag="zs")
                    if has_mlp_b:
                        nc.vector.tensor_scalar_add(
                            out=zs[:], in0=psz[:], scalar1=mbias[:, 1:2])
                    else:
                        nc.vector.tensor_copy(out=zs[:], in_=psz[:])
                    prod = ypool.tile([1, P2], f32, tag="prod")
                    nc.vector.tensor_tensor(
                        out=prod[:], in0=ys[:], in1=zs[:], op=OP.mult)
                    red = ypool.tile([1, 1], f32, tag="red")
                    import concourse.mybir as _mb
                    nc.vector.reduce_sum(red[:], prod[:], axis=_mb.AxisListType.X)
                    nc.scalar.activation(
                        out_sb[:, g:g + 1], red[:], AF.Sigmoid, scale=1.0 / P2)

            nc.sync.dma_start(out=d_out[None, :], in_=out_sb[:1, :])

    nc.compile()
    return nc


def _layout_inputs(feature, W_msg, b_msg, gru_w_ih, gru_w_hh, gru_b_ih, gru_b_hh,
                   conv1_w, conv1_b, conv2_w, conv2_b, convc1_w, convc1_b,
                   convc2_w, convc2_b, mlpy_w, mlpy_b, mlpz_w, mlpz_b,
                   edge_src, edge_dst, edge_type):
    """Host-side sharding + SBUF-layout construction. Index math only
    (plus dtype casts / zero padding / transposes of float inputs)."""
    feature = np.asarray(feature, np.float32)
    edge_src = np.asarray(edge_src).astype(np.int64)
    edge_dst = np.asarray(edge_dst).astype(np.int64)
    edge_type = np.asarray(edge_type).astype(np.int64)

    flags = (
        bool(np.any(np.asarray(b_msg))),
        bool(np.any(np.asarray(gru_b_ih)) or np.any(np.asarray(gru_b_hh))),
        bool(np.any(np.asarray(conv1_b)) or np.any(np.asarray(conv2_b))
             or np.any(np.asarray(convc1_b)) or np.any(np.asarray(convc2_b))),
        bool(np.any(np.asarray(mlpy_b)) or np.any(np.asarray(mlpz_b))),
        bool(USE_FP8_A),
    )
    has_bmsg, has_gru_b, has_conv_b, has_mlp_b, use_fp8 = flags

    # ---- adjacency counts, padded to 640/graph ----
    g_of_e = edge_src // NN
    d_loc = edge_dst - g_of_e * NN
    # packed per-core row index of src: 513*(g mod GPC) + s_loc
    src_packed = edge_src - (g_of_e // GPC) * (GPC * NN)
    kc = src_packed // 128           # packed chunk 0..16 (per core)
    pp = src_packed - kc * 128
    kloc = kc - 4 * (g_of_e % GPC)   # strip chunk 0..4
    # A_h[p, core, g, kloc*T+t, d]
    A_h = np.zeros((128, NCORES, GPC, SC, NN), np.float32)
    np.add.at(A_h, (pp, g_of_e // GPC, g_of_e % GPC,
                    kloc * T + edge_type, d_loc), 1.0)

    # ---- shared weight layouts ----
    W_msg = np.asarray(W_msg, np.float32)          # [T, out, in]
    wmsg_l = np.ascontiguousarray(
        W_msg.transpose(2, 0, 1).reshape(2, 128, T, OUT)
        .transpose(1, 0, 2, 3)).astype(BF16)        # [p, k, t, o]
    wih_l = np.ascontiguousarray(
        np.asarray(gru_w_ih, np.float32).T.reshape(2, 128, 3 * OUT)
        .transpose(1, 0, 2)).astype(BF16)           # [p, k, m]
    whh_l = np.ascontiguousarray(
        np.asarray(gru_w_hh, np.float32).T.reshape(2, 128, 3 * OUT)
        .transpose(1, 0, 2)).astype(BF16)

    def conv_lay(w, nci, nco):
        # w: [cout, cin, k] -> [p, k, ci, co, f]; lhsT[cin_part, cout_free]
        w = np.asarray(w, np.float32)
        k = w.shape[2]
        out = np.zeros((128, k, nci, nco, 128), np.float32)
        for kk in range(k):
            wt = w[:, :, kk].T                      # [cin, cout]
            for ci in range(nci):
                for co in range(nco):
                    out[:, kk, ci, co, :] = wt[ci * 128:(ci + 1) * 128,
                                               co * 128:(co + 1) * 128]
        return out.astype(BF16)

    c1w_l = conv_lay(conv1_w, 2, 2)
    c2w_l = conv_lay(conv2_w, 2, 2)
    cc1w_l = conv_lay(convc1_w, 3, 3)
    cc2w_l = conv_lay(convc2_w, 3, 3)
    mlpy_l = np.ascontiguousarray(
        np.asarray(mlpy_w, np.float32).reshape(2, 128).T).astype(BF16)
    mlpz_l = np.ascontiguousarray(
        np.asarray(mlpz_w, np.float32).reshape(3, 128).T).astype(BF16)

    shared = dict(wmsg=wmsg_l, wih=wih_l, whh=whh_l, c1w=c1w_l, c2w=c2w_l,
                  cc1w=cc1w_l, cc2w=cc2w_l, mlpy=mlpy_l, mlpz=mlpz_l)
    if has_bmsg:
        shared["bmsg"] = np.asarray(b_msg, np.float32)
    if has_gru_b:
        gb = np.zeros((128, 12), np.float32)
        gb[:, 0:6] = np.asarray(gru_b_ih, np.float32).reshape(6, 128).T
        gb[:, 6:12] = np.asarray(gru_b_hh, np.float32).reshape(6, 128).T
        shared["gbias"] = gb
    if has_conv_b:
        cb = np.zeros((128, 10), np.float32)
        cb[:, 0:2] = np.asarray(conv1_b, np.float32).reshape(2, 128).T
        cb[:, 2:4] = np.asarray(conv2_b, np.float32).reshape(2, 128).T
        cb[:, 4:7] = np.asarray(convc1_b, np.float32).reshape(3, 128).T
        cb[:, 7:10] = np.asarray(convc2_b, np.float32).reshape(3, 128).T
        shared["cbias"] = cb
    if has_mlp_b:
        shared["mbias"] = np.array(
            [[float(np.asarray(mlpy_b).reshape(-1)[0]),
              float(np.asarray(mlpz_b).reshape(-1)[0])]], np.float32)

    in_maps = []
    for c in range(NCORES):
        g0 = c * GPC
        feat_l = np.zeros((128, NP), np.float32)
        rows = feature[g0 * NN:(g0 + GPC) * NN]                # [2052, 128]
        feat_l[:, :NV] = rows.T
        A_l = np.ascontiguousarray(A_h[:, c]).astype(
            ml_dtypes.float8_e4m3 if USE_FP8_A else BF16)      # [128,4,20,513]
        m = dict(shared)
        m["feat"] = feat_l.astype(BF16)
        m["adj"] = A_l
        if has_bmsg:
            ind = np.zeros((T, NP), np.float32)
            for g in range(GPC):
                ed_g = (g_of_e // GPC == c) & (g_of_e % GPC == g)
                np.add.at(ind, (edge_type[ed_g], g * NN + d_loc[ed_g]), 1.0)
            m["indeg"] = ind
        in_maps.append(m)
    return flags, in_maps


def kernel(**inputs):
    from concourse.bass_utils import run_bass_kernel_spmd

    flags, in_maps = _layout_inputs(**inputs)
    if flags not in _prog_cache:
        _prog_cache[flags] = _build_program(flags)
    nc = _prog_cache[flags]
    res = run_bass_kernel_spmd(nc, in_maps, core_ids=list(range(NCORES)))
    out = np.concatenate([np.asarray(res.results[c]["out"], np.float32)
                          for c in range(NCORES)])
    return out
